# revision 1
# baseline (speedup 1.0000x reference)
"""DigitCaps dynamic-routing kernel for 8 Trainium2 NeuronCores.

Problem: x(32,16384,8) f32, W(10,16384,8,16) f32 -> v(32,10,16) f32
  u_hat[b,j,p,o] = sum_d x[b,p,d] W[j,p,d,o]   (never materialized!)
  3 routing iterations (softmax over j, weighted sums over p).

Strategy: shard P=16384 over 8 cores (P_loc=2048). Per routing iteration:
  s_part[b,j,o] = sum_{p,d} (c*x)[b,j,p,d] * W[j,p,d,o]     (PE, K=p 128-chunks)
  AllReduce s (20KB) -> v = squash(s)
  z[b,j,p,d]  = sum_o W[j,p,d,o] v[b,j,o]                   (PE, K=(d,o)=128 with
                                                             block-diagonal v rhs)
  uv[b,j,p]   = sum_d x[b,p,d] z[...]                        (DVE, bf16)
  bb += uv ; c = softmax_j(bb)                               (fp32)
Iteration 1 uses c = 0.1 exactly. Final squash + cross-core s-sum on host.
Matmuls run in float32r (TF32-like 1-pass) via AP bitcast; W is streamed
from HBM each phase; the uv-consume path is bf16 on DVE.

Per-core SBUF layouts (p^ = p % 128 on partitions, t = p//128 in 0..15):
  xt  [128, t16, d8, b32]        ws [128, t16, d8, j10, o16]
  wz  [j10, 128=(d*16+o), t16, p128]
"""
import numpy as np
import ml_dtypes
from functools import lru_cache

import concourse.bacc as bacc
import concourse.mybir as mybir
from concourse import tile
from concourse.bass_utils import run_bass_kernel_spmd

F32 = mybir.dt.float32
F32R = mybir.dt.float32r
BF16 = mybir.dt.bfloat16
AX = mybir.AxisListType
ALU = mybir.AluOpType
ACTF = mybir.ActivationFunctionType

B, J, P, D, O = 32, 10, 16384, 8, 16
NCORES = 8
PL = P // NCORES          # 2048
T = PL // 128             # 16 tiles of 128 p's
TG = 4                    # t-group size in z-phase
JO = J * O                # 160


def _emit(nc, n_cores):
    xt = nc.dram_tensor("xt", [128, T, D, B], F32R, kind="ExternalInput")
    xb = nc.dram_tensor("xb", [128, T, D, B], BF16, kind="ExternalInput")
    ws = nc.dram_tensor("ws", [128, T, D, J, O], F32R, kind="ExternalInput")
    wz = nc.dram_tensor("wz", [J, 128, T, 128], F32R, kind="ExternalInput")
    vz = nc.dram_tensor("vz", [128, J, D * B], F32R, kind="ExternalInput")
    s3p = nc.dram_tensor("s3p", [B, JO], F32, kind="ExternalOutput")

    with tile.TileContext(nc) as tc:
        with (
            tc.tile_pool(name="per", bufs=1) as per,        # persistent
            tc.tile_pool(name="wsst", bufs=3) as wsst,      # ws stream
            tc.tile_pool(name="wzst", bufs=3) as wzst,      # wz stream
            tc.tile_pool(name="yp", bufs=2) as yp,
            tc.tile_pool(name="zc", bufs=2) as zc,          # z consume bufs
            tc.tile_pool(name="small", bufs=2) as small,
            tc.tile_pool(name="sps", bufs=2, space="PSUM") as sps,
            tc.tile_pool(name="zps", bufs=2, space="PSUM") as zps,
            tc.tile_pool(name="dram", bufs=2, space="DRAM") as dramp,
        ):
            # warmup collective first: absorbs ncfw's first-collective
            # barrier (~40us) under the iter-0 compute. Contents junk.
            wu_in = dramp.tile([B, 16], F32)
            wu_out = dramp.tile([B, 16], F32)
            wu_sb = small.tile([B, 16], F32)
            nc.gpsimd.memset(wu_sb[:], 0.0)
            nc.sync.dma_start(wu_in[:], wu_sb[:])
            nc.gpsimd.collective_compute(
                "AllReduce", ALU.add,
                replica_groups=[list(range(n_cores))],
                ins=[wu_in[:].opt()], outs=[wu_out[:].opt()],
            )

            x_sb = per.tile([128, T, D, B], F32R)
            nc.sync.dma_start(x_sb[:], xt[:, :, :, :])
            xb_sb = per.tile([128, T, D, B], BF16)
            nc.sync.dma_start(xb_sb[:], xb[:, :, :, :])
            # block-diagonal v holder: rows (d*16+o), cols per j (d*32+b).
            vblk = per.tile([128, J, D * B], F32R)
            nc.sync.dma_start(vblk[:], vz[:, :, :])   # zeros (memset can't f32r)
            bb = per.tile([128, T, J, B], F32)      # routing logits
            e_sb = per.tile([128, T, J, B], F32)    # exp(bb)
            c_sb = per.tile([128, T, J, B], F32R)    # softmax coeffs
            se = per.tile([128, T, B], F32)         # sum_j exp
            rec = per.tile([128, T, B], F32)        # 1/sum

            for it in range(3):
                # ---------------- s-phase ----------------
                s_ps = sps.tile([B, 256], F32)
                if it > 0:
                    # softmax over j: c = exp(bb) / sum_j exp(bb)
                    nc.scalar.activation(e_sb[:], bb[:], ACTF.Exp)
                    nc.vector.tensor_reduce(
                        se[:, :, :, None],
                        e_sb.rearrange("p t j b -> p t b j"),
                        AX.X, ALU.add,
                    )
                    nc.vector.reciprocal(rec[:], se[:])
                    nc.gpsimd.tensor_mul(
                        c_sb[:], e_sb[:],
                        rec[:, :, None, :].broadcast_to([128, T, J, B]),
                    )
                for t in range(T):
                    wst = wsst.tile([128, D, J, O], F32R)
                    nc.sync.dma_start(wst[:], ws[:, t, :, :, :])
                    if it == 0:
                        # c == 0.1 exactly: lhsT = x, scale folded into copy.
                        # N padded 160->256 (reads run into the next d's
                        # region; junk lands in psum cols 160..255, ignored)
                        # to hit fp32r's 1-cycle/row regime; the last chunk
                        # can't overrun the tile so it stays N=160.
                        for d in range(D):
                            rhs = wst.rearrange("p d j o -> p (d j o)")
                            if d == D - 1:  # next-d overrun not possible
                                rhs = rhs[:, d * JO:(d + 1) * JO]
                            else:
                                rhs = rhs[:, d * JO:d * JO + 256]
                            nc.tensor.matmul(
                                s_ps[:, 0:rhs.shape[-1]],
                                x_sb[:, t, d, :],
                                rhs,
                                start=(t == 0 and d == 0),
                                stop=(t == T - 1 and d == D - 1),
                            )
                    else:
                        y_t = yp.tile([128, J, D, B], F32R)
                        # y = c * x, broadcast ops run at 1x -> split the
                        # work between DVE and GpSimd by t parity
                        eng = nc.vector if t % 2 == 0 else nc.gpsimd
                        eng.tensor_mul(
                            y_t[:],
                            c_sb[:, t, :, None, :].broadcast_to([128, J, D, B]),
                            x_sb[:, t, None, :, :].broadcast_to([128, J, D, B]),
                        )
                        for j in range(J):
                            for d in range(D):
                                # single accumulation group per psum bank
                                nc.tensor.matmul(
                                    s_ps[:, j * O:(j + 1) * O],
                                    y_t[:, j, d, :],
                                    wst[:, d, j, :],
                                    start=(t == 0 and j == 0 and d == 0),
                                    stop=(t == T - 1 and j == J - 1 and d == D - 1),
                                )
                s_sb = small.tile([B, JO], F32)
                nc.scalar.activation(s_sb[:], s_ps[:, 0:JO], ACTF.Copy,
                                     scale=0.1 if it == 0 else 1.0)
                if it == 2:
                    nc.sync.dma_start(s3p[:, :], s_sb[:])
                    break

                # ---------------- AllReduce s ----------------
                cc_in = dramp.tile([B, JO], F32)
                cc_out = dramp.tile([B, JO], F32)
                nc.sync.dma_start(cc_in[:], s_sb[:])
                nc.gpsimd.collective_compute(
                    "AllReduce", ALU.add,
                    replica_groups=[list(range(n_cores))],
                    ins=[cc_in[:].opt()], outs=[cc_out[:].opt()],
                )
                s_f = small.tile([B, JO], F32)
                nc.sync.dma_start(s_f[:], cc_out[:])

                # ---------------- squash -> v ----------------
                t2 = small.tile([B, JO], F32)
                nc.vector.tensor_mul(t2[:], s_f[:], s_f[:])
                sq = small.tile([B, J], F32)
                nc.vector.tensor_reduce(
                    sq[:, :, None], t2.rearrange("b (j o) -> b j o", j=J),
                    AX.X, ALU.add)
                r_ = small.tile([B, J], F32)
                nc.scalar.activation(r_[:], sq[:], ACTF.Sqrt)
                den = small.tile([B, J], F32)
                # den = (sq + 1) * r
                nc.vector.scalar_tensor_tensor(
                    den[:], sq[:], 1.0, r_[:], ALU.add, ALU.mult)
                rc2 = small.tile([B, J], F32)
                nc.vector.reciprocal(rc2[:], den[:])
                f_ = small.tile([B, J], F32)
                nc.vector.tensor_mul(f_[:], sq[:], rc2[:])
                v_sb = small.tile([B, J, O], F32R)
                nc.vector.tensor_mul(
                    v_sb[:], s_f.rearrange("b (j o) -> b j o", j=J),
                    f_[:, :, None].broadcast_to([B, J, O]))
                # bounce v through DRAM, then scatter transposed copies into
                # the block-diagonal slots (DMA is exempt from the 32-aligned
                # partition-start rule engine ops have)
                v_dr = dramp.tile([B, J, O], F32R)
                nc.sync.dma_start(v_dr[:], v_sb[:])
                for j in range(J):
                    for d in range(D):
                        nc.sync.dma_start(
                            vblk[d * O:(d + 1) * O, j, d * B:(d + 1) * B],
                            v_dr[:, j, :].rearrange("b o -> o b"))

                # ---------------- z / uv phase ----------------
                for j in range(J):
                    wzs = wzst.tile([128, T, 128], F32R)
                    nc.sync.dma_start(wzs[:], wz[j, :, :, :])
                    for tg in range(T // TG):
                        z_ps = zps.tile([128, TG, D * B], F32)
                        for t4 in range(TG):
                            # two 1KB outputs share each 2KB psum bank ->
                            # pair them into one group per bank
                            nc.tensor.matmul(
                                z_ps[:, t4, :], wzs[:, tg * TG + t4, :],
                                vblk[:, j, :],
                                start=(t4 % 2 == 0), stop=(t4 % 2 == 1))
                        ztmp = zc.tile([128, TG * D * B], BF16)
                        nc.scalar.copy(
                            ztmp[:], z_ps.rearrange("p t db -> p (t db)"))
                        tmp2 = zc.tile([128, TG * D * B], BF16)
                        nc.vector.tensor_mul(
                            tmp2[:], ztmp[:],
                            xb_sb[:, tg * TG:(tg + 1) * TG, :, :]
                            .rearrange("p t d b -> p (t d b)"))
                        t2v = tmp2.rearrange("p (t d b) -> p t d b", t=TG, d=D)
                        u1 = zc.tile([128, TG, 4, B], BF16)
                        nc.vector.tensor_add(
                            u1[:], t2v[:, :, 0:4, :], t2v[:, :, 4:8, :])
                        u2 = zc.tile([128, TG, 2, B], BF16)
                        nc.vector.tensor_add(
                            u2[:], u1[:, :, 0:2, :], u1[:, :, 2:4, :])
                        bb_sl = bb[:, tg * TG:(tg + 1) * TG, j, :]
                        if it == 0:
                            nc.vector.tensor_add(
                                bb_sl, u2[:, :, 0, :], u2[:, :, 1, :])
                        else:
                            uv = zc.tile([128, TG, B], F32)
                            nc.vector.tensor_add(
                                uv[:], u2[:, :, 0, :], u2[:, :, 1, :])
                            nc.vector.tensor_add(bb_sl, bb_sl, uv[:])
    return nc


@lru_cache(maxsize=2)
def _build(n_cores):
    nc = bacc.Bacc("TRN2", target_bir_lowering=False, debug=False,
                   num_devices=n_cores)
    _emit(nc, n_cores)
    nc.compile()
    return nc


def _prep_inputs(x, W):
    """Host-side shard + relayout. Returns list of per-core input dicts."""
    x = np.asarray(x, dtype=np.float32)
    W = np.asarray(W, dtype=np.float32)
    in_maps = []
    for c in range(NCORES):
        xc = x[:, c * PL:(c + 1) * PL, :]              # (B, PL, D)
        Wc = W[:, c * PL:(c + 1) * PL, :, :]           # (J, PL, D, O)
        xr = np.ascontiguousarray(
            xc.reshape(B, T, 128, D).transpose(2, 1, 3, 0))        # [128,T,D,B]
        wsr = np.ascontiguousarray(
            Wc.reshape(J, T, 128, D, O).transpose(2, 1, 3, 0, 4))  # [128,T,D,J,O]
        wzr = np.ascontiguousarray(
            Wc.reshape(J, T, 128, D, O).transpose(0, 3, 4, 1, 2)   # j,d,o,t,p
            .reshape(J, 128, T, 128))                              # [J,(d,o),T,p]
        in_maps.append({"xt": xr, "xb": xr.astype(ml_dtypes.bfloat16),
                        "ws": wsr, "wz": wzr,
                        "vz": np.zeros((128, J, D * B), np.float32)})
    return in_maps


def _squash_np(s):
    sq = np.sum(s * s, axis=-1, keepdims=True)
    return s * (sq / ((1.0 + sq) * np.sqrt(sq)))


def kernel(x, W):
    nc = _build(NCORES)
    in_maps = _prep_inputs(x, W)
    res = run_bass_kernel_spmd(nc, in_maps, list(range(NCORES)))
    s3 = np.zeros((B, JO), np.float64)
    for r in res.results:
        s3 += r["s3p"].astype(np.float64)
    v = _squash_np(s3.reshape(B, J, O))
    return v.astype(np.float32)



# revision 7
# speedup vs baseline: 1.4659x; 1.4659x over previous
"""DigitCaps dynamic-routing kernel for 8 Trainium2 NeuronCores — v2.

Problem: x(32,16384,8) f32, W(10,16384,8,16) f32 -> v(32,10,16) f32
  u_hat[b,j,p,o] = sum_d x[b,p,d] W[j,p,d,o]   (never materialized)
  3 routing iterations (softmax over j, weighted sums over p).

Sharding: P=16384 over 8 cores (PL=2048, T=16 tiles of 128).

Design (vs 592us v1 baseline): W resident in SBUF as bf16 in both
matmul layouts (~11MB/core, loaded once); s-phase uses block-diagonal
matmuls with W stationary [q,(d,o)] and moving yblk [q,(d,b)] N=256
(160 matmuls/iter instead of 1280 tiny f32r ones), accumulating over t
in PSUM per j; the (d,o)x(d,b) diagonal is extracted via a DRAM bounce
into a transposed sT [o,(j,b)] orientation, which makes squash
partition-reducible (gpsimd.partition_all_reduce over o=16) and feeds
the z-phase block-diagonal vblk with 8 DMAs (not 80). z-phase runs
j-outer in two t-half passes so softmax for the next iteration of
half 0 overlaps z of half 1; the consume path is fp16: PSUM->SBUF
copy split Act/DVE, fused mul + add-tree on DVE, bb add on gpsimd.

Per-core SBUF layouts (q = p % 128 on partitions, t = p//128 in 0..15):
  wsb [q, j, t, (d*16+o)]  bf16   s-phase stationary tiles
  wzb [(d*16+o), j, t, q]  bf16   z-phase stationary tiles
  xb/xh [q, t, d, b]       bf16/fp16
  bb  [q, t, j, b]         f32    routing logits
"""
import numpy as np
import ml_dtypes
from functools import lru_cache

import concourse.bacc as bacc
import concourse.mybir as mybir
from concourse import tile
from concourse import bass_isa
from concourse.bass_utils import run_bass_kernel_spmd

F32 = mybir.dt.float32
BF16 = mybir.dt.bfloat16
FP16 = mybir.dt.float16
AX = mybir.AxisListType
ALU = mybir.AluOpType
ACTF = mybir.ActivationFunctionType

B, J, P, D, O = 32, 10, 16384, 8, 16
NCORES = 8
PL = P // NCORES          # 2048
T = PL // 128             # 16
DO = D * O                # 128
DB = D * B                # 256
JB = J * B                # 320
TH = T // 2               # t-half size (8)


def _emit(nc, n_cores):
    wsd = nc.dram_tensor("wsb", [128, J, T, DO], BF16, kind="ExternalInput")
    wzd = nc.dram_tensor("wzb", [DO, J, T, 128], BF16, kind="ExternalInput")
    xbd = nc.dram_tensor("xb", [128, T, D, B], BF16, kind="ExternalInput")
    s3p = nc.dram_tensor("s3p", [O, J, B], F32, kind="ExternalOutput")

    with tile.TileContext(nc) as tc:
        with (
            tc.tile_pool(name="per", bufs=1) as per,
            tc.tile_pool(name="yp", bufs=2) as yp,          # yblk chunks
            tc.tile_pool(name="zb", bufs=2) as zbp,         # z sbuf fp16
            tc.tile_pool(name="tm", bufs=2) as tmp_p,       # consume mul out
            tc.tile_pool(name="tr", bufs=2) as tree_p,      # tree temps
            tc.tile_pool(name="red", bufs=1) as red,
            tc.tile_pool(name="small", bufs=1) as small,
            tc.tile_pool(name="sps", bufs=2, space="PSUM") as sps,
            tc.tile_pool(name="zps", bufs=2, space="PSUM") as zps,
            tc.tile_pool(name="dram", bufs=6, space="DRAM") as dramp,
        ):
            # warmup collective: absorbs ncfw's first-collective barrier
            # (~40us) under the load + iter-0 compute. Contents junk.
            wu_in = dramp.tile([B, 16], F32)
            wu_out = dramp.tile([B, 16], F32)
            wu_sb = small.tile([B, 16], F32)
            nc.gpsimd.memset(wu_sb[:], 0.0)
            nc.sync.dma_start(wu_in[:], wu_sb[:])
            nc.gpsimd.collective_compute(
                "AllReduce", ALU.add,
                replica_groups=[list(range(n_cores))],
                ins=[wu_in[:].opt()], outs=[wu_out[:].opt()],
            )

            # ---------------- resident loads ----------------
            xb = per.tile([128, T, D, B], BF16)
            nc.sync.dma_start(xb[:], xbd[:, :, :, :])
            wsb = per.tile([128, J, T, DO], BF16)
            for j in range(J):
                nc.sync.dma_start(wsb[:, j, :, :], wsd[:, j, :, :])
            wzb = per.tile([128, J, T, 128], BF16)
            for j in range(J):
                nc.sync.dma_start(wzb[:, j, :, :], wzd[:, j, :, :])

            # ---------------- persistent state ----------------
            bb = per.tile([128, T, J, B], F32)       # routing logits
            e_sb = per.tile([128, T, J, B], BF16)    # exp(bb)
            se = per.tile([128, T, B], F32)          # sum_j exp
            recb = per.tile([128, T, B], BF16)
            xp = per.tile([128, T, D, B], BF16)      # x * rec
            vblk = per.tile([128, J, DB], BF16)      # block-diag v
            nc.gpsimd.memset(vblk[:], 0.0)
            s_all = per.tile([128, J, DB], FP16)     # s-psum staging
            ext = per.tile([16, D, J, B], FP16)      # extract staging
            sT = per.tile([16, J, B], F32)           # s partial [o,(j,b)]

            def s_phase(it):
                """s-matmuls for iteration `it` -> sT (pre-AllReduce)."""
                for j in range(J):
                    s_ps = sps.tile([128, 512], F32)     # full psum bank
                    for th in range(2):
                        if it > 0:
                            yb = yp.tile([128, TH, D, B], BF16)
                            nc.vector.tensor_mul(
                                yb[:],
                                e_sb[:, th * TH:(th + 1) * TH, j, None, :]
                                .broadcast_to([128, TH, D, B]),
                                xp[:, th * TH:(th + 1) * TH, :, :],
                            )
                        for ti in range(TH):
                            t = th * TH + ti
                            rhs = (xb[:, t, :, :] if it == 0 else
                                   yb[:, ti, :, :]).rearrange("q d b -> q (d b)")
                            nc.tensor.matmul(
                                s_ps[:, 0:DB],
                                wsb[:, j, t, :],
                                rhs,
                                start=(t == 0), stop=(t == T - 1),
                            )
                    nc.scalar.copy(s_all[:, j, :], s_ps[:, 0:DB])
                # diagonal extraction: sT[o,j,b] = sum_d s_all[d*16+o, j, d*32+b]
                ext_dr = dramp.tile([D, 16, J, B], FP16)
                for d in range(D):
                    nc.sync.dma_start(
                        ext_dr[d, :, :, :],
                        s_all[d * 16:(d + 1) * 16, :, d * B:(d + 1) * B])
                nc.sync.dma_start(
                    ext[:], ext_dr.rearrange("d o j b -> o d j b"))
                a1 = red.tile([16, 2, J, B], F32)
                nc.vector.tensor_add(a1[:], ext[:, 0:2, :, :], ext[:, 2:4, :, :])
                a2 = red.tile([16, 2, J, B], F32)
                nc.vector.tensor_add(a2[:], ext[:, 4:6, :, :], ext[:, 6:8, :, :])
                a3 = red.tile([16, 2, J, B], F32)
                nc.vector.tensor_add(a3[:], a1[:], a2[:])
                if it == 0:
                    # c == 0.1 exactly in iteration 0
                    st0 = small.tile([16, J, B], F32)
                    nc.vector.tensor_add(st0[:], a3[:, 0, :, :], a3[:, 1, :, :])
                    nc.vector.tensor_scalar_mul(sT[:], st0[:], 0.1)
                else:
                    nc.vector.tensor_add(sT[:], a3[:, 0, :, :], a3[:, 1, :, :])

            def allreduce_squash():
                """sT -> AllReduce -> squash -> v [o,(j,b)] bf16 -> vblk."""
                cc_in = dramp.tile([16, JB], F32)
                cc_out = dramp.tile([16, JB], F32)
                nc.sync.dma_start(cc_in[:], sT.rearrange("o j b -> o (j b)"))
                nc.gpsimd.collective_compute(
                    "AllReduce", ALU.add,
                    replica_groups=[list(range(n_cores))],
                    ins=[cc_in[:].opt()], outs=[cc_out[:].opt()],
                )
                sf = small.tile([16, J, B], F32)
                nc.sync.dma_start(sf[:], cc_out.rearrange("o (j b) -> o j b", j=J))
                t2 = small.tile([16, J, B], F32)
                nc.vector.tensor_mul(t2[:], sf[:], sf[:])
                sq = small.tile([16, J, B], F32)
                nc.gpsimd.partition_all_reduce(
                    sq[:], t2[:], channels=16, reduce_op=bass_isa.ReduceOp.add)
                r_ = small.tile([16, J, B], F32)
                nc.scalar.activation(r_[:], sq[:], ACTF.Sqrt)
                den = small.tile([16, J, B], F32)
                nc.vector.scalar_tensor_tensor(
                    den[:], sq[:], 1.0, r_[:], ALU.add, ALU.mult)
                rc2 = small.tile([16, J, B], F32)
                nc.vector.reciprocal(rc2[:], den[:])
                f_ = small.tile([16, J, B], F32)
                nc.vector.tensor_mul(f_[:], sq[:], rc2[:])
                v_sb = small.tile([16, J, B], BF16)
                nc.vector.tensor_mul(v_sb[:], sf[:], f_[:])
                # scatter into block-diagonal vblk via DRAM bounce
                v_dr = dramp.tile([16, J, B], BF16)
                nc.sync.dma_start(v_dr[:], v_sb[:])
                for d in range(D):
                    nc.sync.dma_start(
                        vblk[d * 16:(d + 1) * 16, :, d * B:(d + 1) * B],
                        v_dr[:, :, :])

            def softmax_half(th):
                """exp/sum/rec/xp for t-half `th` of bb (feeds next s)."""
                sl = slice(th * TH, (th + 1) * TH)
                nc.scalar.activation(e_sb[:, sl, :, :], bb[:, sl, :, :], ACTF.Exp)
                nc.vector.tensor_reduce(
                    se[:, sl, :, None],
                    e_sb[:, sl, :, :].rearrange("q t j b -> q t b j"),
                    AX.X, ALU.add)
                with nc.allow_low_precision(reason="softmax denom, bf16 ok"):
                    nc.vector.reciprocal(recb[:, sl, :], se[:, sl, :])
                nc.vector.tensor_mul(
                    xp[:, sl, :, :],
                    xb[:, sl, :, :],
                    recb[:, sl, None, :].broadcast_to([128, TH, D, B]))

            def z_phase(it):
                """z-matmuls + consume: bb[...] (+)= sum_d x*z; j-outer,
                two t-half passes; softmax of the half overlaps next pass."""
                for th in range(2):
                    for j in range(J):
                        zb = zbp.tile([128, TH, D, B], BF16)
                        for tc4 in range(TH // 4):
                            z_ps = zps.tile([128, 4, DB], F32)
                            for ti in range(4):
                                t = th * TH + tc4 * 4 + ti
                                nc.tensor.matmul(
                                    z_ps[:, ti, :],
                                    wzb[:, j, t, :],
                                    vblk[:, j, :],
                                    start=(ti % 2 == 0), stop=(ti % 2 == 1),
                                )
                            # split the f32->fp16 copy across Act and DVE
                            dst = zb[:, tc4 * 4:tc4 * 4 + 4, :, :]\
                                .rearrange("q t d b -> q (t d b)")
                            src = z_ps.rearrange("q t db -> q (t db)")
                            nc.scalar.copy(dst[:, 0:640], src[:, 0:640])
                            nc.vector.tensor_scalar_mul(
                                dst[:, 640:1024], src[:, 640:1024], 1.0)
                        tmp = tmp_p.tile([128, TH, D, B], BF16)
                        nc.vector.tensor_mul(
                            tmp[:], zb[:],
                            xb[:, th * TH:(th + 1) * TH, :, :])
                        u1 = tree_p.tile([128, TH, 4, B], BF16)
                        nc.vector.tensor_add(
                            u1[:], tmp[:, :, 0:4, :], tmp[:, :, 4:8, :])
                        u2 = tree_p.tile([128, TH, 2, B], BF16)
                        nc.vector.tensor_add(
                            u2[:], u1[:, :, 0:2, :], u1[:, :, 2:4, :])
                        bb_sl = bb[:, th * TH:(th + 1) * TH, j, :]
                        if it == 0:
                            nc.vector.tensor_add(
                                bb_sl, u2[:, :, 0, :], u2[:, :, 1, :])
                        else:
                            uv = tree_p.tile([128, TH, B], BF16)
                            nc.vector.tensor_add(
                                uv[:], u2[:, :, 0, :], u2[:, :, 1, :])
                            nc.gpsimd.tensor_add(bb_sl, bb_sl, uv[:])
                    softmax_half(th)

            # ---------------- main flow ----------------
            s_phase(0)
            allreduce_squash()
            z_phase(0)              # + softmax halves for iter 1
            s_phase(1)
            allreduce_squash()
            z_phase(1)              # + softmax halves for iter 2
            s_phase(2)
            nc.sync.dma_start(s3p[:, :, :], sT[:])
    return nc


@lru_cache(maxsize=2)
def _build(n_cores):
    nc = bacc.Bacc("TRN2", target_bir_lowering=False, debug=False,
                   num_devices=n_cores)
    _emit(nc, n_cores)
    nc.compile()
    return nc


def _prep_inputs(x, W):
    """Host-side shard + relayout. Returns list of per-core input dicts."""
    x = np.asarray(x, dtype=np.float32)
    W = np.asarray(W, dtype=np.float32)
    in_maps = []
    for c in range(NCORES):
        xc = x[:, c * PL:(c + 1) * PL, :]              # (B, PL, D)
        Wc = W[:, c * PL:(c + 1) * PL, :, :]           # (J, PL, D, O)
        w5 = Wc.reshape(J, T, 128, D, O)
        wsb = np.ascontiguousarray(
            w5.transpose(2, 0, 1, 3, 4).reshape(128, J, T, DO)
        ).astype(ml_dtypes.bfloat16)
        wzb = np.ascontiguousarray(
            w5.transpose(3, 4, 0, 1, 2).reshape(DO, J, T, 128)
        ).astype(ml_dtypes.bfloat16)
        xbr = np.ascontiguousarray(
            xc.reshape(B, T, 128, D).transpose(2, 1, 3, 0))  # [q, t, d, b]
        in_maps.append({"wsb": wsb, "wzb": wzb,
                        "xb": xbr.astype(ml_dtypes.bfloat16)})
    return in_maps


def _squash_np(s):
    sq = np.sum(s * s, axis=-1, keepdims=True)
    return s * (sq / ((1.0 + sq) * np.sqrt(sq)))


def kernel(x, W):
    nc = _build(NCORES)
    in_maps = _prep_inputs(x, W)
    res = run_bass_kernel_spmd(nc, in_maps, list(range(NCORES)))
    s3 = np.zeros((O, J, B), np.float64)
    for r in res.results:
        s3 += r["s3p"].astype(np.float64)
    v = _squash_np(s3.transpose(2, 1, 0))          # [b, j, o]
    return v.astype(np.float32)


# revision 10
# speedup vs baseline: 1.6411x; 1.1195x over previous
"""DigitCaps dynamic-routing kernel for 8 Trainium2 NeuronCores — v3.

Problem: x(32,16384,8) f32, W(10,16384,8,16) f32 -> v(32,10,16) f32
  u_hat[b,j,p,o] = sum_d x[b,p,d] W[j,p,d,o]   (never materialized)
  3 routing iterations (softmax over j, weighted sums over p).

Sharding: P=16384 over 8 cores (PL=2048, T=16 tiles of 128).

Design: W resident in SBUF as bf16 in both matmul layouts (~11MB/core,
loaded once); s-phase uses block-diagonal matmuls with W stationary
[q,(d,o)] and moving yblk [q,(d,b)] N=256 (160 matmuls/iter),
accumulating over t in PSUM per j; the (d,o)x(d,b) diagonal is
extracted via DMA into a transposed sT [o,(j,b)] orientation, which
makes squash a 16-partition reduction (ones-matmul on PE that also
broadcasts) and feeds the z-phase block-diagonal vblk with 8 DMAs.
z-phase runs j-outer in two t-half passes so softmax for the next
iteration of half 0 overlaps z of half 1. Consume: PSUM->SBUF bf16
copies on Act+GpSimd (DVE stays free for the mul + add-tree, whose
tails are batched across j-pairs). Final iteration outputs the raw
staged s (host does diagonal extraction + squash).
"""
import numpy as np
import ml_dtypes
from functools import lru_cache

import concourse.bacc as bacc
import concourse.mybir as mybir
from concourse import tile
from concourse.bass_utils import run_bass_kernel_spmd

F32 = mybir.dt.float32
BF16 = mybir.dt.bfloat16
FP16 = mybir.dt.float16
AX = mybir.AxisListType
ALU = mybir.AluOpType
ACTF = mybir.ActivationFunctionType

B, J, P, D, O = 32, 10, 16384, 8, 16
NCORES = 8
PL = P // NCORES          # 2048
T = PL // 128             # 16
DO = D * O                # 128
DB = D * B                # 256
JB = J * B                # 320
TH = T // 2               # t-half size (8)


def _emit(nc, n_cores):
    wsd = nc.dram_tensor("wsb", [128, J, T, DO], BF16, kind="ExternalInput")
    wzd = nc.dram_tensor("wzb", [DO, J, T, 128], BF16, kind="ExternalInput")
    xbd = nc.dram_tensor("xb", [128, T, D, B], BF16, kind="ExternalInput")
    s3p = nc.dram_tensor("s3p", [128, J, DB], FP16, kind="ExternalOutput")

    with tile.TileContext(nc) as tc:
        with (
            tc.tile_pool(name="per", bufs=1) as per,
            tc.tile_pool(name="yp", bufs=2) as yp,          # yblk chunks
            tc.tile_pool(name="zb", bufs=2) as zbp,         # z sbuf bf16
            tc.tile_pool(name="tm", bufs=2) as tmp_p,       # consume mul out
            tc.tile_pool(name="tr", bufs=2) as tree_p,      # tree temps
            tc.tile_pool(name="sx", bufs=1) as sxp,         # softmax temps
            tc.tile_pool(name="red", bufs=1) as red,
            tc.tile_pool(name="small", bufs=1) as small,
            tc.tile_pool(name="sps", bufs=2, space="PSUM") as sps,
            tc.tile_pool(name="qps", bufs=2, space="PSUM") as qps,
            tc.tile_pool(name="zps", bufs=2, space="PSUM") as zps,
            tc.tile_pool(name="dram", bufs=6, space="DRAM") as dramp,
        ):
            # warmup collective FIRST with no upstream deps (junk data):
            # absorbs ncfw's first-collective barrier (~36us) under the
            # resident loads + iter-0 compute.
            wu_in = dramp.tile([B, 16], F32)
            wu_out = dramp.tile([B, 16], F32)
            nc.gpsimd.collective_compute(
                "AllReduce", ALU.add,
                replica_groups=[list(range(n_cores))],
                ins=[wu_in[:].opt()], outs=[wu_out[:].opt()],
            )

            # ---------------- resident loads ----------------
            xb = per.tile([128, T, D, B], BF16)
            nc.sync.dma_start(xb[:], xbd[:, :, :, :])
            wsb = per.tile([128, J, T, DO], BF16)
            for j in range(J):
                nc.sync.dma_start(wsb[:, j, :, :], wsd[:, j, :, :])
            wzb = per.tile([128, J, T, 128], BF16)
            for j in range(J):
                nc.sync.dma_start(wzb[:, j, :, :], wzd[:, j, :, :])

            # ---------------- persistent state ----------------
            bb = per.tile([128, T, J, B], F32)       # routing logits
            e_sb = per.tile([128, T, J, B], BF16)    # exp(bb)
            recb = per.tile([128, T, B], BF16)       # 1/sum_j exp
            xp = per.tile([128, T, D, B], BF16)      # x * rec
            vblk = per.tile([128, J, DB], BF16)      # block-diag v
            nc.gpsimd.memset(vblk[:], 0.0)
            ones16 = per.tile([16, 16], F32)         # squash reduce weights
            nc.gpsimd.memset(ones16[:], 1.0)
            s_all = per.tile([128, J, DB], FP16)     # s-psum staging
            ext = per.tile([16, D, J, B], FP16)      # extract staging
            sT = per.tile([16, J, B], F32)           # s partial [o,(j,b)]

            def s_phase(it):
                """s-matmuls for iteration `it` -> s_all (+ sT if it<2)."""
                for j in range(J):
                    s_ps = sps.tile([128, 512], F32)     # full psum bank
                    for th in range(2):
                        if it > 0:
                            yb = yp.tile([128, TH, D, B], BF16)
                            nc.vector.tensor_mul(
                                yb[:],
                                e_sb[:, th * TH:(th + 1) * TH, j, None, :]
                                .broadcast_to([128, TH, D, B]),
                                xp[:, th * TH:(th + 1) * TH, :, :],
                            )
                        for ti in range(TH):
                            t = th * TH + ti
                            rhs = (xb[:, t, :, :] if it == 0 else
                                   yb[:, ti, :, :]).rearrange("q d b -> q (d b)")
                            nc.tensor.matmul(
                                s_ps[:, 0:DB],
                                wsb[:, j, t, :],
                                rhs,
                                start=(t == 0), stop=(t == T - 1),
                            )
                    nc.scalar.copy(s_all[:, j, :], s_ps[:, 0:DB])
                if it == 2:
                    nc.sync.dma_start(s3p[:, :, :], s_all[:])
                    return
                # diagonal extraction: sT[o,j,b] = sum_d s_all[d*16+o, j, d*32+b]
                for d in range(D):
                    nc.sync.dma_start(
                        ext[:, d, :, :],
                        s_all[d * 16:(d + 1) * 16, :, d * B:(d + 1) * B])
                a1 = red.tile([16, 2, J, B], F32)
                nc.vector.tensor_add(a1[:], ext[:, 0:2, :, :], ext[:, 2:4, :, :])
                a2 = red.tile([16, 2, J, B], F32)
                nc.vector.tensor_add(a2[:], ext[:, 4:6, :, :], ext[:, 6:8, :, :])
                a3 = red.tile([16, 2, J, B], F32)
                nc.vector.tensor_add(a3[:], a1[:], a2[:])
                if it == 0:
                    # c == 0.1 exactly in iteration 0
                    st0 = small.tile([16, J, B], F32)
                    nc.vector.tensor_add(st0[:], a3[:, 0, :, :], a3[:, 1, :, :])
                    nc.vector.tensor_scalar_mul(sT[:], st0[:], 0.1)
                else:
                    nc.vector.tensor_add(sT[:], a3[:, 0, :, :], a3[:, 1, :, :])

            def allreduce_squash():
                """sT -> AllReduce -> squash -> v [o,(j,b)] bf16 -> vblk."""
                cc_in = dramp.tile([16, JB], F32)
                cc_out = dramp.tile([16, JB], F32)
                nc.sync.dma_start(cc_in[:], sT.rearrange("o j b -> o (j b)"))
                nc.gpsimd.collective_compute(
                    "AllReduce", ALU.add,
                    replica_groups=[list(range(n_cores))],
                    ins=[cc_in[:].opt()], outs=[cc_out[:].opt()],
                )
                sf = small.tile([16, J, B], F32)
                nc.sync.dma_start(sf[:], cc_out.rearrange("o (j b) -> o j b", j=J))
                t2 = small.tile([16, J, B], F32)
                nc.vector.tensor_mul(t2[:], sf[:], sf[:])
                # sq = sum_o t2, replicated to all 16 partitions via PE
                sq_ps = qps.tile([16, JB], F32)
                nc.tensor.matmul(
                    sq_ps[:], ones16[:], t2.rearrange("o j b -> o (j b)"),
                    start=True, stop=True)
                r_ = small.tile([16, J, B], F32)
                nc.scalar.activation(
                    r_[:], sq_ps.rearrange("o (j b) -> o j b", j=J), ACTF.Sqrt)
                den = small.tile([16, J, B], F32)
                nc.vector.tensor_scalar_add(
                    den[:], sq_ps.rearrange("o (j b) -> o j b", j=J), 1.0)
                # v = sf * sqrt(sq)/(1+sq)
                rc = small.tile([16, J, B], F32)
                nc.vector.reciprocal(rc[:], den[:])
                f_ = small.tile([16, J, B], F32)
                nc.vector.tensor_mul(f_[:], r_[:], rc[:])
                v_sb = small.tile([16, J, B], BF16)
                nc.vector.tensor_mul(v_sb[:], sf[:], f_[:])
                # scatter into block-diagonal vblk (SBUF->SBUF DMAs)
                for d in range(D):
                    nc.sync.dma_start(
                        vblk[d * 16:(d + 1) * 16, :, d * B:(d + 1) * B],
                        v_sb[:, :, :])

            def softmax_half(th):
                """exp/sum/rec/xp for t-half `th` of bb (feeds next s)."""
                sl = slice(th * TH, (th + 1) * TH)
                nc.scalar.activation(e_sb[:, sl, :, :], bb[:, sl, :, :], ACTF.Exp)
                # se = sum_j e via j-split add tree (bf16 partials)
                sA = sxp.tile([128, TH, 5, B], BF16)
                nc.vector.tensor_add(
                    sA[:], e_sb[:, sl, 0:5, :], e_sb[:, sl, 5:10, :])
                sB = sxp.tile([128, TH, 2, B], BF16)
                nc.vector.tensor_add(sB[:], sA[:, :, 0:2, :], sA[:, :, 2:4, :])
                sC = sxp.tile([128, TH, B], BF16)
                nc.vector.tensor_add(sC[:], sB[:, :, 0, :], sB[:, :, 1, :])
                se = sxp.tile([128, TH, B], F32)
                nc.vector.tensor_add(se[:], sC[:], sA[:, :, 4, :])
                with nc.allow_low_precision(reason="softmax denom, bf16 ok"):
                    nc.vector.reciprocal(recb[:, sl, :], se[:])
                nc.vector.tensor_mul(
                    xp[:, sl, :, :],
                    xb[:, sl, :, :],
                    recb[:, sl, None, :].broadcast_to([128, TH, D, B]))

            def z_phase(it):
                """z-matmuls + consume: bb[...] (+)= sum_d x*z; j-outer,
                two t-half passes; softmax of the half overlaps next pass."""
                gp_rounds = (4, 9) if it == 0 else (4,)
                for th in range(2):
                    u1a = None
                    for j in range(J):
                        zb = zbp.tile([128, TH, D, B], BF16)
                        cpy_dve = (j == 2)
                        meng = nc.gpsimd if j in gp_rounds else nc.vector
                        for tc4 in range(TH // 4):
                            z_ps = zps.tile([128, 4, DB], F32)
                            for ti in range(4):
                                t = th * TH + tc4 * 4 + ti
                                nc.tensor.matmul(
                                    z_ps[:, ti, :],
                                    wzb[:, j, t, :],
                                    vblk[:, j, :],
                                    start=(ti % 2 == 0), stop=(ti % 2 == 1),
                                )
                            dst = zb[:, tc4 * 4:tc4 * 4 + 4, :, :]\
                                .rearrange("q t d b -> q (t d b)")
                            zsrc = z_ps.rearrange("q t db -> q (t db)")
                            if cpy_dve:
                                nc.vector.tensor_scalar_mul(dst[:], zsrc[:], 1.0)
                            else:
                                nc.scalar.copy(dst[:], zsrc[:])
                        tmp = tmp_p.tile([128, TH, D, B], BF16)
                        meng.tensor_mul(
                            tmp[:], zb[:],
                            xb[:, th * TH:(th + 1) * TH, :, :])
                        # L1 per j into a j-pair batched buffer
                        if j % 2 == 0:
                            u1a = tree_p.tile([128, 2, TH, 4, B], BF16)
                        meng.tensor_add(
                            u1a[:, j % 2, :, :, :],
                            tmp[:, :, 0:4, :], tmp[:, :, 4:8, :])
                        if j % 2 == 1:
                            u2 = tree_p.tile([128, 2, TH, 2, B], BF16)
                            nc.vector.tensor_add(
                                u2[:], u1a[:, :, :, 0:2, :], u1a[:, :, :, 2:4, :])
                            bb_sl = bb[:, th * TH:(th + 1) * TH, j - 1:j + 1, :]\
                                .rearrange("q t j b -> q j t b")
                            if it == 0:
                                nc.vector.tensor_add(
                                    bb_sl, u2[:, :, :, 0, :], u2[:, :, :, 1, :])
                            else:
                                uv = tree_p.tile([128, 2, TH, B], BF16)
                                nc.vector.tensor_add(
                                    uv[:], u2[:, :, :, 0, :], u2[:, :, :, 1, :])
                                nc.gpsimd.tensor_add(bb_sl, bb_sl, uv[:])
                    softmax_half(th)

            # ---------------- main flow ----------------
            s_phase(0)
            allreduce_squash()
            z_phase(0)              # + softmax halves for iter 1
            s_phase(1)
            allreduce_squash()
            z_phase(1)              # + softmax halves for iter 2
            s_phase(2)              # writes raw s_all to s3p
    return nc


@lru_cache(maxsize=2)
def _build(n_cores):
    nc = bacc.Bacc("TRN2", target_bir_lowering=False, debug=False,
                   num_devices=n_cores)
    _emit(nc, n_cores)
    nc.compile()
    return nc


def _prep_inputs(x, W):
    """Host-side shard + relayout. Returns list of per-core input dicts."""
    x = np.asarray(x, dtype=np.float32)
    W = np.asarray(W, dtype=np.float32)
    in_maps = []
    for c in range(NCORES):
        xc = x[:, c * PL:(c + 1) * PL, :]              # (B, PL, D)
        Wc = W[:, c * PL:(c + 1) * PL, :, :]           # (J, PL, D, O)
        w5 = Wc.reshape(J, T, 128, D, O)
        wsb = np.ascontiguousarray(
            w5.transpose(2, 0, 1, 3, 4).reshape(128, J, T, DO)
        ).astype(ml_dtypes.bfloat16)
        wzb = np.ascontiguousarray(
            w5.transpose(3, 4, 0, 1, 2).reshape(DO, J, T, 128)
        ).astype(ml_dtypes.bfloat16)
        xbr = np.ascontiguousarray(
            xc.reshape(B, T, 128, D).transpose(2, 1, 3, 0))  # [q, t, d, b]
        in_maps.append({"wsb": wsb, "wzb": wzb,
                        "xb": xbr.astype(ml_dtypes.bfloat16)})
    return in_maps


def _squash_np(s):
    sq = np.sum(s * s, axis=-1, keepdims=True)
    return s * (sq / ((1.0 + sq) * np.sqrt(sq)))


def _extract_sT(s_raw):
    """Host diagonal extraction: [128, J, DB] -> [O, J, B]."""
    out = np.zeros((O, J, B), np.float64)
    for d in range(D):
        out += s_raw[d * O:(d + 1) * O, :, d * B:(d + 1) * B]
    return out


def kernel(x, W):
    nc = _build(NCORES)
    in_maps = _prep_inputs(x, W)
    res = run_bass_kernel_spmd(nc, in_maps, list(range(NCORES)))
    s3 = np.zeros((O, J, B), np.float64)
    for r in res.results:
        s3 += _extract_sT(r["s3p"].astype(np.float64))
    v = _squash_np(s3.transpose(2, 1, 0))          # [b, j, o]
    return v.astype(np.float32)


# revision 13
# speedup vs baseline: 1.6915x; 1.0307x over previous
"""DigitCaps dynamic-routing kernel for 8 Trainium2 NeuronCores — v3.

Problem: x(32,16384,8) f32, W(10,16384,8,16) f32 -> v(32,10,16) f32
  u_hat[b,j,p,o] = sum_d x[b,p,d] W[j,p,d,o]   (never materialized)
  3 routing iterations (softmax over j, weighted sums over p).

Sharding: P=16384 over 8 cores (PL=2048, T=16 tiles of 128).

Design: W resident in SBUF as bf16 in both matmul layouts (~11MB/core,
loaded once); s-phase uses block-diagonal matmuls with W stationary
[q,(d,o)] and moving yblk [q,(d,b)] N=256 (160 matmuls/iter),
accumulating over t in PSUM per j; the (d,o)x(d,b) diagonal is
extracted via DMA into a transposed sT [o,(j,b)] orientation, which
makes squash a 16-partition reduction (ones-matmul on PE that also
broadcasts) and feeds the z-phase block-diagonal vblk with 8 DMAs.
z-phase runs j-outer in two t-half passes so softmax for the next
iteration of half 0 overlaps z of half 1. Consume: PSUM->SBUF bf16
copies on Act+GpSimd (DVE stays free for the mul + add-tree, whose
tails are batched across j-pairs). Final iteration outputs the raw
staged s (host does diagonal extraction + squash).
"""
import numpy as np
import ml_dtypes
from functools import lru_cache

import concourse.bacc as bacc
import concourse.mybir as mybir
from concourse import tile
from concourse.bass_utils import run_bass_kernel_spmd

F32 = mybir.dt.float32
BF16 = mybir.dt.bfloat16
FP16 = mybir.dt.float16
AX = mybir.AxisListType
ALU = mybir.AluOpType
ACTF = mybir.ActivationFunctionType

B, J, P, D, O = 32, 10, 16384, 8, 16
NCORES = 8
PL = P // NCORES          # 2048
T = PL // 128             # 16
DO = D * O                # 128
DB = D * B                # 256
JB = J * B                # 320
TH = T // 2               # t-half size (8)


def _emit(nc, n_cores):
    wsd = nc.dram_tensor("wsb", [128, J, T, DO], BF16, kind="ExternalInput")
    wzd = nc.dram_tensor("wzb", [DO, J, T, 128], BF16, kind="ExternalInput")
    xbd = nc.dram_tensor("xb", [128, T, D, B], BF16, kind="ExternalInput")
    s3p = nc.dram_tensor("s3p", [128, J, DB], FP16, kind="ExternalOutput")

    with tile.TileContext(nc) as tc:
        with (
            tc.tile_pool(name="per", bufs=1) as per,
            tc.tile_pool(name="yp", bufs=2) as yp,          # yblk chunks
            tc.tile_pool(name="zb", bufs=2) as zbp,         # z sbuf bf16
            tc.tile_pool(name="tm", bufs=2) as tmp_p,       # consume mul out
            tc.tile_pool(name="tr", bufs=2) as tree_p,      # tree temps
            tc.tile_pool(name="sx", bufs=1) as sxp,         # softmax temps
            tc.tile_pool(name="red", bufs=1) as red,
            tc.tile_pool(name="small", bufs=1) as small,
            tc.tile_pool(name="sps", bufs=2, space="PSUM") as sps,
            tc.tile_pool(name="qps", bufs=2, space="PSUM") as qps,
            tc.tile_pool(name="zps", bufs=2, space="PSUM") as zps,
            tc.tile_pool(name="dram", bufs=6, space="DRAM") as dramp,
        ):
            # warmup collective FIRST with no upstream deps (junk data):
            # absorbs ncfw's first-collective barrier (~36us) under the
            # resident loads + iter-0 compute.
            wu_in = dramp.tile([B, 16], F32)
            wu_out = dramp.tile([B, 16], F32)
            nc.gpsimd.collective_compute(
                "AllReduce", ALU.add,
                replica_groups=[list(range(n_cores))],
                ins=[wu_in[:].opt()], outs=[wu_out[:].opt()],
            )

            # ---------------- resident loads ----------------
            xb = per.tile([128, T, D, B], BF16)
            nc.sync.dma_start(xb[:], xbd[:, :, :, :])
            wsb = per.tile([128, J, T, DO], BF16)
            for j in range(J):
                nc.sync.dma_start(wsb[:, j, :, :], wsd[:, j, :, :])
            wzb = per.tile([128, J, T, 128], BF16)   # loaded inside z0

            # ---------------- persistent state ----------------
            bb = per.tile([128, T, J, B], F32)       # routing logits
            e_sb = per.tile([128, T, J, B], BF16)    # exp(bb)
            recb = per.tile([128, T, B], BF16)       # 1/sum_j exp
            xp = per.tile([128, T, D, B], BF16)      # x * rec
            vblk = per.tile([128, J, DB], BF16)      # block-diag v
            nc.gpsimd.memset(vblk[:], 0.0)
            ones16 = per.tile([16, 16], F32)         # squash reduce weights
            nc.gpsimd.memset(ones16[:], 1.0)
            s_all = per.tile([128, J, DB], FP16)     # s-psum staging
            ext = per.tile([16, D, J, B], FP16)      # extract staging
            sT = per.tile([16, J, B], F32)           # s partial [o,(j,b)]

            def s_phase(it):
                """s-matmuls for iteration `it` -> s_all (+ sT if it<2)."""
                for j in range(J):
                    s_ps = sps.tile([128, 512], F32)     # full psum bank
                    for th in range(2):
                        if it > 0:
                            yb = yp.tile([128, TH, D, B], BF16)
                            nc.vector.tensor_mul(
                                yb[:],
                                e_sb[:, th * TH:(th + 1) * TH, j, None, :]
                                .broadcast_to([128, TH, D, B]),
                                xp[:, th * TH:(th + 1) * TH, :, :],
                            )
                        for ti in range(TH):
                            t = th * TH + ti
                            rhs = (xb[:, t, :, :] if it == 0 else
                                   yb[:, ti, :, :]).rearrange("q d b -> q (d b)")
                            nc.tensor.matmul(
                                s_ps[:, 0:DB],
                                wsb[:, j, t, :],
                                rhs,
                                start=(t == 0), stop=(t == T - 1),
                            )
                    nc.scalar.copy(s_all[:, j, :], s_ps[:, 0:DB])
                if it == 2:
                    nc.sync.dma_start(s3p[:, :, :], s_all[:])
                    return
                # diagonal extraction: sT[o,j,b] = sum_d s_all[d*16+o, j, d*32+b]
                for d in range(D):
                    nc.sync.dma_start(
                        ext[:, d, :, :],
                        s_all[d * 16:(d + 1) * 16, :, d * B:(d + 1) * B])
                a1 = red.tile([16, 2, J, B], F32)
                nc.vector.tensor_add(a1[:], ext[:, 0:2, :, :], ext[:, 2:4, :, :])
                a2 = red.tile([16, 2, J, B], F32)
                nc.vector.tensor_add(a2[:], ext[:, 4:6, :, :], ext[:, 6:8, :, :])
                a3 = red.tile([16, 2, J, B], F32)
                nc.vector.tensor_add(a3[:], a1[:], a2[:])
                if it == 0:
                    # c == 0.1 exactly in iteration 0
                    st0 = small.tile([16, J, B], F32)
                    nc.vector.tensor_add(st0[:], a3[:, 0, :, :], a3[:, 1, :, :])
                    nc.vector.tensor_scalar_mul(sT[:], st0[:], 0.1)
                else:
                    nc.vector.tensor_add(sT[:], a3[:, 0, :, :], a3[:, 1, :, :])

            def allreduce_squash():
                """sT -> AllReduce -> squash -> v [o,(j,b)] bf16 -> vblk."""
                cc_in = dramp.tile([16, JB], F32)
                cc_out = dramp.tile([16, JB], F32)
                nc.sync.dma_start(cc_in[:], sT.rearrange("o j b -> o (j b)"))
                nc.gpsimd.collective_compute(
                    "AllReduce", ALU.add,
                    replica_groups=[list(range(n_cores))],
                    ins=[cc_in[:].opt()], outs=[cc_out[:].opt()],
                )
                sf = small.tile([16, J, B], F32)
                nc.sync.dma_start(sf[:], cc_out.rearrange("o (j b) -> o j b", j=J))
                t2 = small.tile([16, J, B], F32)
                nc.vector.tensor_mul(t2[:], sf[:], sf[:])
                # sq = sum_o t2, replicated to all 16 partitions via PE
                sq_ps = qps.tile([16, JB], F32)
                nc.tensor.matmul(
                    sq_ps[:], ones16[:], t2.rearrange("o j b -> o (j b)"),
                    start=True, stop=True)
                r_ = small.tile([16, J, B], F32)
                nc.scalar.activation(
                    r_[:], sq_ps.rearrange("o (j b) -> o j b", j=J), ACTF.Sqrt)
                den = small.tile([16, J, B], F32)
                nc.vector.tensor_scalar_add(
                    den[:], sq_ps.rearrange("o (j b) -> o j b", j=J), 1.0)
                # v = sf * sqrt(sq)/(1+sq)
                rc = small.tile([16, J, B], F32)
                nc.vector.reciprocal(rc[:], den[:])
                f_ = small.tile([16, J, B], F32)
                nc.vector.tensor_mul(f_[:], r_[:], rc[:])
                v_sb = small.tile([16, J, B], BF16)
                nc.vector.tensor_mul(v_sb[:], sf[:], f_[:])
                # scatter into block-diagonal vblk (SBUF->SBUF DMAs)
                for d in range(D):
                    nc.sync.dma_start(
                        vblk[d * 16:(d + 1) * 16, :, d * B:(d + 1) * B],
                        v_sb[:, :, :])

            def softmax_half(th):
                """exp/sum/rec/xp for t-half `th` of bb (feeds next s)."""
                sl = slice(th * TH, (th + 1) * TH)
                nc.scalar.activation(e_sb[:, sl, :, :], bb[:, sl, :, :], ACTF.Exp)
                # se = sum_j e via j-split add tree (bf16 partials)
                sA = sxp.tile([128, TH, 5, B], BF16)
                nc.vector.tensor_add(
                    sA[:], e_sb[:, sl, 0:5, :], e_sb[:, sl, 5:10, :])
                sB = sxp.tile([128, TH, 2, B], BF16)
                nc.vector.tensor_add(sB[:], sA[:, :, 0:2, :], sA[:, :, 2:4, :])
                sC = sxp.tile([128, TH, B], BF16)
                nc.vector.tensor_add(sC[:], sB[:, :, 0, :], sB[:, :, 1, :])
                se = sxp.tile([128, TH, B], F32)
                nc.vector.tensor_add(se[:], sC[:], sA[:, :, 4, :])
                with nc.allow_low_precision(reason="softmax denom, bf16 ok"):
                    nc.vector.reciprocal(recb[:, sl, :], se[:])
                nc.vector.tensor_mul(
                    xp[:, sl, :, :],
                    xb[:, sl, :, :],
                    recb[:, sl, None, :].broadcast_to([128, TH, D, B]))

            def z_phase(it):
                """z-matmuls + consume: bb[...] (+)= sum_d x*z; j-outer,
                two t-half passes; softmax of the half overlaps next pass."""
                for th in range(2):
                    u1a = None
                    for j in range(J):
                        if it == 0 and th == 0:
                            nc.sync.dma_start(wzb[:, j, :, :], wzd[:, j, :, :])
                        direct = j in (2, 8)
                        zb = None if direct else zbp.tile([128, TH, D, B], BF16)
                        tmp = tmp_p.tile([128, TH, D, B], BF16)
                        for tc4 in range(TH // 4):
                            z_ps = zps.tile([128, 4, DB], F32)
                            for ti in range(4):
                                t = th * TH + tc4 * 4 + ti
                                nc.tensor.matmul(
                                    z_ps[:, ti, :],
                                    wzb[:, j, t, :],
                                    vblk[:, j, :],
                                    start=(ti % 2 == 0), stop=(ti % 2 == 1),
                                )
                            c4 = slice(tc4 * 4, tc4 * 4 + 4)
                            if direct:
                                # fused psum-read mul, skips the copy
                                nc.vector.tensor_mul(
                                    tmp[:, c4, :, :].rearrange(
                                        "q t d b -> q (t d b)"),
                                    z_ps.rearrange("q t db -> q (t db)"),
                                    xb[:, th * TH + tc4 * 4:
                                       th * TH + tc4 * 4 + 4, :, :].rearrange(
                                        "q t d b -> q (t d b)"))
                            else:
                                nc.scalar.copy(
                                    zb[:, c4, :, :].rearrange(
                                        "q t d b -> q (t d b)"),
                                    z_ps.rearrange("q t db -> q (t db)"))
                        if not direct:
                            nc.vector.tensor_mul(
                                tmp[:], zb[:],
                                xb[:, th * TH:(th + 1) * TH, :, :])
                        # L1 per j into a j-pair batched buffer
                        if j % 2 == 0:
                            u1a = tree_p.tile([128, 2, TH, 4, B], BF16)
                        nc.vector.tensor_add(
                            u1a[:, j % 2, :, :, :],
                            tmp[:, :, 0:4, :], tmp[:, :, 4:8, :])
                        if j % 2 == 1:
                            u2 = tree_p.tile([128, 2, TH, 2, B], BF16)
                            nc.vector.tensor_add(
                                u2[:], u1a[:, :, :, 0:2, :], u1a[:, :, :, 2:4, :])
                            bb_sl = bb[:, th * TH:(th + 1) * TH, j - 1:j + 1, :]\
                                .rearrange("q t j b -> q j t b")
                            if it == 0:
                                nc.vector.tensor_add(
                                    bb_sl, u2[:, :, :, 0, :], u2[:, :, :, 1, :])
                            else:
                                uv = tree_p.tile([128, 2, TH, B], BF16)
                                nc.vector.tensor_add(
                                    uv[:], u2[:, :, :, 0, :], u2[:, :, :, 1, :])
                                nc.gpsimd.tensor_add(bb_sl, bb_sl, uv[:])
                    softmax_half(th)

            # ---------------- main flow ----------------
            s_phase(0)
            allreduce_squash()
            z_phase(0)              # + softmax halves for iter 1
            s_phase(1)
            allreduce_squash()
            z_phase(1)              # + softmax halves for iter 2
            s_phase(2)              # writes raw s_all to s3p
    return nc


@lru_cache(maxsize=2)
def _build(n_cores):
    nc = bacc.Bacc("TRN2", target_bir_lowering=False, debug=False,
                   num_devices=n_cores)
    _emit(nc, n_cores)
    nc.compile()
    return nc


def _prep_inputs(x, W):
    """Host-side shard + relayout. Returns list of per-core input dicts."""
    x = np.asarray(x, dtype=np.float32)
    W = np.asarray(W, dtype=np.float32)
    in_maps = []
    for c in range(NCORES):
        xc = x[:, c * PL:(c + 1) * PL, :]              # (B, PL, D)
        Wc = W[:, c * PL:(c + 1) * PL, :, :]           # (J, PL, D, O)
        w5 = Wc.reshape(J, T, 128, D, O)
        wsb = np.ascontiguousarray(
            w5.transpose(2, 0, 1, 3, 4).reshape(128, J, T, DO)
        ).astype(ml_dtypes.bfloat16)
        wzb = np.ascontiguousarray(
            w5.transpose(3, 4, 0, 1, 2).reshape(DO, J, T, 128)
        ).astype(ml_dtypes.bfloat16)
        xbr = np.ascontiguousarray(
            xc.reshape(B, T, 128, D).transpose(2, 1, 3, 0))  # [q, t, d, b]
        in_maps.append({"wsb": wsb, "wzb": wzb,
                        "xb": xbr.astype(ml_dtypes.bfloat16)})
    return in_maps


def _squash_np(s):
    sq = np.sum(s * s, axis=-1, keepdims=True)
    return s * (sq / ((1.0 + sq) * np.sqrt(sq)))


def _extract_sT(s_raw):
    """Host diagonal extraction: [128, J, DB] -> [O, J, B]."""
    out = np.zeros((O, J, B), np.float64)
    for d in range(D):
        out += s_raw[d * O:(d + 1) * O, :, d * B:(d + 1) * B]
    return out


def kernel(x, W):
    nc = _build(NCORES)
    in_maps = _prep_inputs(x, W)
    res = run_bass_kernel_spmd(nc, in_maps, list(range(NCORES)))
    s3 = np.zeros((O, J, B), np.float64)
    for r in res.results:
        s3 += _extract_sT(r["s3p"].astype(np.float64))
    v = _squash_np(s3.transpose(2, 1, 0))          # [b, j, o]
    return v.astype(np.float32)


# revision 14
# speedup vs baseline: 1.6985x; 1.0042x over previous
"""DigitCaps dynamic-routing kernel for 8 Trainium2 NeuronCores — v3.

Problem: x(32,16384,8) f32, W(10,16384,8,16) f32 -> v(32,10,16) f32
  u_hat[b,j,p,o] = sum_d x[b,p,d] W[j,p,d,o]   (never materialized)
  3 routing iterations (softmax over j, weighted sums over p).

Sharding: P=16384 over 8 cores (PL=2048, T=16 tiles of 128).

Design: W resident in SBUF as bf16 in both matmul layouts (~11MB/core,
loaded once); s-phase uses block-diagonal matmuls with W stationary
[q,(d,o)] and moving yblk [q,(d,b)] N=256 (160 matmuls/iter),
accumulating over t in PSUM per j; the (d,o)x(d,b) diagonal is
extracted via DMA into a transposed sT [o,(j,b)] orientation, which
makes squash a 16-partition reduction (ones-matmul on PE that also
broadcasts) and feeds the z-phase block-diagonal vblk with 8 DMAs.
z-phase runs j-outer in two t-half passes so softmax for the next
iteration of half 0 overlaps z of half 1. Consume: PSUM->SBUF bf16
copies on Act+GpSimd (DVE stays free for the mul + add-tree, whose
tails are batched across j-pairs). Final iteration outputs the raw
staged s (host does diagonal extraction + squash).
"""
import numpy as np
import ml_dtypes
from functools import lru_cache

import concourse.bacc as bacc
import concourse.mybir as mybir
from concourse import tile
from concourse.bass_utils import run_bass_kernel_spmd

F32 = mybir.dt.float32
BF16 = mybir.dt.bfloat16
FP16 = mybir.dt.float16
AX = mybir.AxisListType
ALU = mybir.AluOpType
ACTF = mybir.ActivationFunctionType

B, J, P, D, O = 32, 10, 16384, 8, 16
NCORES = 8
PL = P // NCORES          # 2048
T = PL // 128             # 16
DO = D * O                # 128
DB = D * B                # 256
JB = J * B                # 320
TH = T // 2               # t-half size (8)


def _emit(nc, n_cores):
    wsd = nc.dram_tensor("wsb", [128, J, T, DO], BF16, kind="ExternalInput")
    wzd = nc.dram_tensor("wzb", [DO, J, T, 128], BF16, kind="ExternalInput")
    xbd = nc.dram_tensor("xb", [128, T, D, B], BF16, kind="ExternalInput")
    s3p = nc.dram_tensor("s3p", [128, J, DB], FP16, kind="ExternalOutput")

    with tile.TileContext(nc) as tc:
        with (
            tc.tile_pool(name="per", bufs=1) as per,
            tc.tile_pool(name="yp", bufs=2) as yp,          # yblk chunks
            tc.tile_pool(name="zb", bufs=2) as zbp,         # z sbuf bf16
            tc.tile_pool(name="tm", bufs=2) as tmp_p,       # consume mul out
            tc.tile_pool(name="tr", bufs=2) as tree_p,      # tree temps
            tc.tile_pool(name="sx", bufs=1) as sxp,         # softmax temps
            tc.tile_pool(name="red", bufs=1) as red,
            tc.tile_pool(name="small", bufs=1) as small,
            tc.tile_pool(name="sps", bufs=2, space="PSUM") as sps,
            tc.tile_pool(name="qps", bufs=2, space="PSUM") as qps,
            tc.tile_pool(name="zps", bufs=2, space="PSUM") as zps,
            tc.tile_pool(name="dram", bufs=6, space="DRAM") as dramp,
        ):
            # warmup collective FIRST with no upstream deps (junk data):
            # absorbs ncfw's first-collective barrier (~36us) under the
            # resident loads + iter-0 compute.
            wu_in = dramp.tile([B, 16], F32)
            wu_out = dramp.tile([B, 16], F32)
            nc.gpsimd.collective_compute(
                "AllReduce", ALU.add,
                replica_groups=[list(range(n_cores))],
                ins=[wu_in[:].opt()], outs=[wu_out[:].opt()],
            )

            # ---------------- resident loads ----------------
            xb = per.tile([128, T, D, B], BF16)
            nc.sync.dma_start(xb[:], xbd[:, :, :, :])
            wsb = per.tile([128, J, T, DO], BF16)
            for j in range(J):
                nc.sync.dma_start(wsb[:, j, :, :], wsd[:, j, :, :])
            wzb = per.tile([128, J, T, 128], BF16)   # loaded inside z0

            # ---------------- persistent state ----------------
            bb = per.tile([128, T, J, B], F32)       # routing logits
            e_sb = per.tile([128, T, J, B], BF16)    # exp(bb)
            recb = per.tile([128, T, B], BF16)       # 1/sum_j exp
            xp = per.tile([128, T, D, B], BF16)      # x * rec
            vblk = per.tile([128, J, DB], BF16)      # block-diag v
            nc.vector.memset(vblk[:], 0.0)
            ones16 = per.tile([16, 16], F32)         # squash reduce weights
            nc.vector.memset(ones16[:], 1.0)
            s_all = per.tile([128, J, DB], FP16)     # s-psum staging
            ext = per.tile([16, D, J, B], FP16)      # extract staging
            sT = per.tile([16, J, B], F32)           # s partial [o,(j,b)]

            def s_phase(it):
                """s-matmuls for iteration `it` -> s_all (+ sT if it<2)."""
                for j in range(J):
                    s_ps = sps.tile([128, 512], F32)     # full psum bank
                    for th in range(2):
                        if it > 0:
                            yb = yp.tile([128, TH, D, B], BF16)
                            nc.vector.tensor_mul(
                                yb[:],
                                e_sb[:, th * TH:(th + 1) * TH, j, None, :]
                                .broadcast_to([128, TH, D, B]),
                                xp[:, th * TH:(th + 1) * TH, :, :],
                            )
                        for ti in range(TH):
                            t = th * TH + ti
                            rhs = (xb[:, t, :, :] if it == 0 else
                                   yb[:, ti, :, :]).rearrange("q d b -> q (d b)")
                            nc.tensor.matmul(
                                s_ps[:, 0:DB],
                                wsb[:, j, t, :],
                                rhs,
                                start=(t == 0), stop=(t == T - 1),
                            )
                    nc.scalar.copy(s_all[:, j, :], s_ps[:, 0:DB])
                    if it == 2:
                        nc.sync.dma_start(s3p[:, j, :], s_all[:, j, :])
                if it == 2:
                    return
                # diagonal extraction: sT[o,j,b] = sum_d s_all[d*16+o, j, d*32+b]
                for d in range(D):
                    nc.sync.dma_start(
                        ext[:, d, :, :],
                        s_all[d * 16:(d + 1) * 16, :, d * B:(d + 1) * B])
                a1 = red.tile([16, 2, J, B], F32)
                nc.vector.tensor_add(a1[:], ext[:, 0:2, :, :], ext[:, 2:4, :, :])
                a2 = red.tile([16, 2, J, B], F32)
                nc.vector.tensor_add(a2[:], ext[:, 4:6, :, :], ext[:, 6:8, :, :])
                a3 = red.tile([16, 2, J, B], F32)
                nc.vector.tensor_add(a3[:], a1[:], a2[:])
                if it == 0:
                    # c == 0.1 exactly in iteration 0
                    st0 = small.tile([16, J, B], F32)
                    nc.vector.tensor_add(st0[:], a3[:, 0, :, :], a3[:, 1, :, :])
                    nc.vector.tensor_scalar_mul(sT[:], st0[:], 0.1)
                else:
                    nc.vector.tensor_add(sT[:], a3[:, 0, :, :], a3[:, 1, :, :])

            def allreduce_squash():
                """sT -> AllReduce -> squash -> v [o,(j,b)] bf16 -> vblk."""
                cc_in = dramp.tile([16, JB], F32)
                cc_out = dramp.tile([16, JB], F32)
                nc.sync.dma_start(cc_in[:], sT.rearrange("o j b -> o (j b)"))
                nc.gpsimd.collective_compute(
                    "AllReduce", ALU.add,
                    replica_groups=[list(range(n_cores))],
                    ins=[cc_in[:].opt()], outs=[cc_out[:].opt()],
                )
                sf = small.tile([16, J, B], F32)
                nc.sync.dma_start(sf[:], cc_out.rearrange("o (j b) -> o j b", j=J))
                t2 = small.tile([16, J, B], F32)
                nc.vector.tensor_mul(t2[:], sf[:], sf[:])
                # sq = sum_o t2, replicated to all 16 partitions via PE
                sq_ps = qps.tile([16, JB], F32)
                nc.tensor.matmul(
                    sq_ps[:], ones16[:], t2.rearrange("o j b -> o (j b)"),
                    start=True, stop=True)
                r_ = small.tile([16, J, B], F32)
                nc.scalar.activation(
                    r_[:], sq_ps.rearrange("o (j b) -> o j b", j=J), ACTF.Sqrt)
                den = small.tile([16, J, B], F32)
                nc.vector.tensor_scalar_add(
                    den[:], sq_ps.rearrange("o (j b) -> o j b", j=J), 1.0)
                # v = sf * sqrt(sq)/(1+sq)
                rc = small.tile([16, J, B], F32)
                nc.vector.reciprocal(rc[:], den[:])
                f_ = small.tile([16, J, B], F32)
                nc.vector.tensor_mul(f_[:], r_[:], rc[:])
                v_sb = small.tile([16, J, B], BF16)
                nc.vector.tensor_mul(v_sb[:], sf[:], f_[:])
                # scatter into block-diagonal vblk (SBUF->SBUF DMAs)
                for d in range(D):
                    nc.sync.dma_start(
                        vblk[d * 16:(d + 1) * 16, :, d * B:(d + 1) * B],
                        v_sb[:, :, :])

            def softmax_half(th):
                """exp/sum/rec/xp for t-half `th` of bb (feeds next s)."""
                sl = slice(th * TH, (th + 1) * TH)
                nc.scalar.activation(e_sb[:, sl, :, :], bb[:, sl, :, :], ACTF.Exp)
                # se = sum_j e via j-split add tree (bf16 partials)
                sA = sxp.tile([128, TH, 5, B], BF16)
                nc.vector.tensor_add(
                    sA[:], e_sb[:, sl, 0:5, :], e_sb[:, sl, 5:10, :])
                sB = sxp.tile([128, TH, 2, B], BF16)
                nc.vector.tensor_add(sB[:], sA[:, :, 0:2, :], sA[:, :, 2:4, :])
                sC = sxp.tile([128, TH, B], BF16)
                nc.vector.tensor_add(sC[:], sB[:, :, 0, :], sB[:, :, 1, :])
                se = sxp.tile([128, TH, B], F32)
                nc.vector.tensor_add(se[:], sC[:], sA[:, :, 4, :])
                with nc.allow_low_precision(reason="softmax denom, bf16 ok"):
                    nc.vector.reciprocal(recb[:, sl, :], se[:])
                nc.vector.tensor_mul(
                    xp[:, sl, :, :],
                    xb[:, sl, :, :],
                    recb[:, sl, None, :].broadcast_to([128, TH, D, B]))

            def z_phase(it):
                """z-matmuls + consume: bb[...] (+)= sum_d x*z; j-outer,
                two t-half passes; softmax of the half overlaps next pass."""
                for th in range(2):
                    u1a = None
                    for j in range(J):
                        if it == 0 and th == 0:
                            nc.sync.dma_start(wzb[:, j, :, :], wzd[:, j, :, :])
                        direct = j in (2, 8)
                        zb = None if direct else zbp.tile([128, TH, D, B], BF16)
                        tmp = tmp_p.tile([128, TH, D, B], BF16)
                        for tc4 in range(TH // 4):
                            z_ps = zps.tile([128, 4, DB], F32)
                            for ti in range(4):
                                t = th * TH + tc4 * 4 + ti
                                nc.tensor.matmul(
                                    z_ps[:, ti, :],
                                    wzb[:, j, t, :],
                                    vblk[:, j, :],
                                    start=(ti % 2 == 0), stop=(ti % 2 == 1),
                                )
                            c4 = slice(tc4 * 4, tc4 * 4 + 4)
                            if direct:
                                # fused psum-read mul, skips the copy
                                nc.vector.tensor_mul(
                                    tmp[:, c4, :, :].rearrange(
                                        "q t d b -> q (t d b)"),
                                    z_ps.rearrange("q t db -> q (t db)"),
                                    xb[:, th * TH + tc4 * 4:
                                       th * TH + tc4 * 4 + 4, :, :].rearrange(
                                        "q t d b -> q (t d b)"))
                            else:
                                nc.scalar.copy(
                                    zb[:, c4, :, :].rearrange(
                                        "q t d b -> q (t d b)"),
                                    z_ps.rearrange("q t db -> q (t db)"))
                        if not direct:
                            nc.vector.tensor_mul(
                                tmp[:], zb[:],
                                xb[:, th * TH:(th + 1) * TH, :, :])
                        # L1 per j into a j-pair batched buffer
                        if j % 2 == 0:
                            u1a = tree_p.tile([128, 2, TH, 4, B], BF16)
                        nc.vector.tensor_add(
                            u1a[:, j % 2, :, :, :],
                            tmp[:, :, 0:4, :], tmp[:, :, 4:8, :])
                        if j % 2 == 1:
                            u2 = tree_p.tile([128, 2, TH, 2, B], BF16)
                            nc.vector.tensor_add(
                                u2[:], u1a[:, :, :, 0:2, :], u1a[:, :, :, 2:4, :])
                            bb_sl = bb[:, th * TH:(th + 1) * TH, j - 1:j + 1, :]\
                                .rearrange("q t j b -> q j t b")
                            if it == 0:
                                nc.vector.tensor_add(
                                    bb_sl, u2[:, :, :, 0, :], u2[:, :, :, 1, :])
                            else:
                                uv = tree_p.tile([128, 2, TH, B], BF16)
                                nc.vector.tensor_add(
                                    uv[:], u2[:, :, :, 0, :], u2[:, :, :, 1, :])
                                nc.gpsimd.tensor_add(bb_sl, bb_sl, uv[:])
                    softmax_half(th)

            # ---------------- main flow ----------------
            s_phase(0)
            allreduce_squash()
            z_phase(0)              # + softmax halves for iter 1
            s_phase(1)
            allreduce_squash()
            z_phase(1)              # + softmax halves for iter 2
            s_phase(2)              # writes raw s_all to s3p
    return nc


@lru_cache(maxsize=2)
def _build(n_cores):
    nc = bacc.Bacc("TRN2", target_bir_lowering=False, debug=False,
                   num_devices=n_cores)
    _emit(nc, n_cores)
    nc.compile()
    return nc


def _prep_inputs(x, W):
    """Host-side shard + relayout. Returns list of per-core input dicts."""
    x = np.asarray(x, dtype=np.float32)
    W = np.asarray(W, dtype=np.float32)
    in_maps = []
    for c in range(NCORES):
        xc = x[:, c * PL:(c + 1) * PL, :]              # (B, PL, D)
        Wc = W[:, c * PL:(c + 1) * PL, :, :]           # (J, PL, D, O)
        w5 = Wc.reshape(J, T, 128, D, O)
        wsb = np.ascontiguousarray(
            w5.transpose(2, 0, 1, 3, 4).reshape(128, J, T, DO)
        ).astype(ml_dtypes.bfloat16)
        wzb = np.ascontiguousarray(
            w5.transpose(3, 4, 0, 1, 2).reshape(DO, J, T, 128)
        ).astype(ml_dtypes.bfloat16)
        xbr = np.ascontiguousarray(
            xc.reshape(B, T, 128, D).transpose(2, 1, 3, 0))  # [q, t, d, b]
        in_maps.append({"wsb": wsb, "wzb": wzb,
                        "xb": xbr.astype(ml_dtypes.bfloat16)})
    return in_maps


def _squash_np(s):
    sq = np.sum(s * s, axis=-1, keepdims=True)
    return s * (sq / ((1.0 + sq) * np.sqrt(sq)))


def _extract_sT(s_raw):
    """Host diagonal extraction: [128, J, DB] -> [O, J, B]."""
    out = np.zeros((O, J, B), np.float64)
    for d in range(D):
        out += s_raw[d * O:(d + 1) * O, :, d * B:(d + 1) * B]
    return out


def kernel(x, W):
    nc = _build(NCORES)
    in_maps = _prep_inputs(x, W)
    res = run_bass_kernel_spmd(nc, in_maps, list(range(NCORES)))
    s3 = np.zeros((O, J, B), np.float64)
    for r in res.results:
        s3 += _extract_sT(r["s3p"].astype(np.float64))
    v = _squash_np(s3.transpose(2, 1, 0))          # [b, j, o]
    return v.astype(np.float32)
